# revision 1
# baseline (speedup 1.0000x reference)
"""Trainium2 Bass kernel for nn_NodeModel (GNN message passing + external
attention + MLP), SPMD across 8 NeuronCores.

Sharding: nodes (and their incoming edges) are partitioned by destination-node
range across the 8 cores; small params are replicated. Host pre-sorts edges by
destination 128-node window; on-device segment_sum is one matmul per 128-edge
chunk (one-hot edge->node selection stationary, edge features moving),
accumulating agg [128 nodes, HID] in PSUM.

LayerNorm gamma/beta are folded into the attention/MLP weights host-side:
  scores = xhat @ (gamma*Mk).T + Mk@beta
  h = relu(attn @ (a*Mv@W1 + 1⊗b1p) + xhat @ ((1-a)*gamma*W1))
      (b1p rides on Mv1 because softmax rows sum to 1)
  y = h @ W2 + b2
so the device only computes xhat = (cat - mean) / sqrt(var + eps).
"""

import sys

if "/opt/trn_rl_repo" not in sys.path:
    sys.path.insert(0, "/opt/trn_rl_repo")

import numpy as np

N, E, V_IN, HID, U_IN, B, MEM = 50000, 800000, 128, 128, 64, 64, 128
CAT = V_IN + HID + U_IN  # 320
ALPHA = 0.5
EPS = 1e-5
NCORES = 8
P = 128
N_LOC = N // NCORES        # 6250 nodes per core
NW = (N_LOC + P - 1) // P  # 49 windows of 128 nodes
N_PAD = NW * P             # 6272
OHK = 1                    # one-hot chunks built per DVE op


# ---------------------------------------------------------------------------
# Workarounds for this container's walrus: at most ONE sync wait per
# instruction is encodable. Tile's scheduler emits multi-waits; split them
# onto same-engine NoOps. Same for the TileContext exit drain.
# ---------------------------------------------------------------------------

def _patched_drain_and_barrier(self, tick_clock, wait_clock):
    from concourse.vector_clock import ScopedClock, VectorClock

    nc = self.nc
    gvc = tick_clock.global_clock
    nprocs = len(gvc)
    for proc in range(nprocs):
        tick = gvc[proc]
        if tick <= 0:
            continue
        one = VectorClock([0] * nprocs)
        one.require_at_least(proc, tick)
        inst = nc.sync.drain()
        wait_clock.add_sem_waits(inst.ins, ScopedClock({None: one}))
    nc.sync.drain()
    nc.all_engine_barrier()
    assert self.sems is not None
    popped = nc._tile_sem_poison_stack.pop()
    assert popped is self._sem_poison
    nc.clear_and_free_semaphores(list(self.sems.allocated().values()))
    nc.all_engine_barrier()


def _split_multi_waits(nc):
    from concourse import mybir

    for f in nc.m.functions:
        for bb in f.blocks:
            out = []
            for inst in bb.instructions:
                si = inst.sync_info
                if si is not None and si.on_wait is not None and len(si.on_wait) > 1:
                    waits = list(si.on_wait)
                    for i, w in enumerate(waits[:-1]):
                        out.append(mybir.InstNoOp(
                            name=f"{inst.name}-wsplit{i}",
                            engine=inst.engine,
                            sync_info=mybir.SyncInfo(on_wait=[w], on_update=[]),
                        ))
                    si.on_wait = waits[-1:]
                out.append(inst)
            bb.instructions[:] = out


_patch_applied = False


def _apply_patches():
    global _patch_applied
    if _patch_applied:
        return
    import concourse.tile as tile

    tile.TileContext._drain_and_barrier = _patched_drain_and_barrier
    _patch_applied = True


# ---------------------------------------------------------------------------
# Bass module builder. Kernel structure depends only on the per-window chunk
# counts C (shared across cores), so cache on that.
# ---------------------------------------------------------------------------

_nc_cache = {}

# Edge payload mode:
#   "bf16"  — single bf16 copy: halves the dominant HBM traffic, seg matmuls
#             at 1 cy/row (vs fp32's 4), but ~4e-3 relative error.
#   "split" — bf16 hi + bf16 lo residual: same bytes as fp32, seg matmuls
#             2 cy/row effective, ~1e-5 relative error.
EDGE_MODE = "split"
EDGE_BF16 = EDGE_MODE in ("bf16", "split")
EHALVES = 2 if EDGE_MODE == "split" else 1


def _build(key, split_waits=True):
    """key: (C, sb_zero, b2_zero); C = per-window 128-edge chunk counts."""
    import concourse.bass as bass
    import concourse.tile as tile
    from concourse import mybir

    C, sb_zero, b2_zero = key
    _apply_patches()
    f32 = mybir.dt.float32
    f32r = mybir.dt.float32r
    edt = mybir.dt.bfloat16 if EDGE_BF16 else f32
    Cmax = max(C)
    Cmax4 = ((Cmax + OHK - 1) // OHK) * OHK
    E_pad = sum(C) * P

    nc = bass.Bass()
    d_ea = nc.dram_tensor("ea", [E_pad * EHALVES * HID], edt, kind="ExternalInput")
    d_dstl = nc.dram_tensor("dstl", [E_pad], f32, kind="ExternalInput")
    d_x = nc.dram_tensor("x", [N_PAD, V_IN], f32, kind="ExternalInput")
    d_ub = nc.dram_tensor("ub", [N_PAD, U_IN], f32, kind="ExternalInput")
    d_mw = nc.dram_tensor("mw", [CAT, 2 * P], f32, kind="ExternalInput")
    d_mv1 = nc.dram_tensor("mv1", [MEM, HID], f32, kind="ExternalInput")
    d_w2 = nc.dram_tensor("w2", [HID, HID], f32, kind="ExternalInput")
    d_sb = nc.dram_tensor("sb", [1, MEM], f32, kind="ExternalInput")
    d_b2 = nc.dram_tensor("b2", [1, HID], f32, kind="ExternalInput")
    d_iota = nc.dram_tensor("iota", [P, OHK, P], edt, kind="ExternalInput")
    d_id = nc.dram_tensor("ident", [P, P], f32, kind="ExternalInput")
    d_out = nc.dram_tensor("out", [N_PAD, HID], f32, kind="ExternalOutput")

    KCH = [(0, 0, 128), (1, 128, 128), (2, 256, 64)]  # (j, cat offset, K)

    with tile.TileContext(nc) as tc:
        with (
            tc.tile_pool(name="const", bufs=1) as cpool,
            tc.tile_pool(name="edges", bufs=3) as epool,
            tc.tile_pool(name="oh", bufs=3) as ohpool,
            tc.tile_pool(name="cat", bufs=4) as catpool,
            tc.tile_pool(name="xt", bufs=3) as xtpool,
            tc.tile_pool(name="small", bufs=8) as spool,
            tc.tile_pool(name="work", bufs=3) as wpool,
            tc.tile_pool(name="agg_ps", bufs=3, space="PSUM") as aggps,
            tc.tile_pool(name="tr_ps", bufs=2, space="PSUM") as trps,
            tc.tile_pool(name="mm_ps", bufs=3, space="PSUM") as mmps,
        ):
            # constants
            t_mw = cpool.tile([P, 3, 2 * P], f32)   # [ MkgT | W1g ] per K chunk
            for j, off, K in KCH:
                nc.sync.dma_start(out=t_mw[:K, j, :], in_=d_mw[off:off + K, :])
            t_mv1 = cpool.tile([P, P], f32)
            nc.sync.dma_start(out=t_mv1[:], in_=d_mv1[:])
            t_w2 = cpool.tile([P, P], f32)
            nc.sync.dma_start(out=t_w2[:], in_=d_w2[:])
            if not sb_zero:
                t_sb = cpool.tile([1, P], f32)
                nc.sync.dma_start(out=t_sb[:1], in_=d_sb[:])
            if not b2_zero:
                t_b2 = cpool.tile([1, P], f32)
                nc.sync.dma_start(out=t_b2[:1], in_=d_b2[:])
            t_iota = cpool.tile([P, OHK, P], edt)
            nc.sync.dma_start(out=t_iota[:], in_=d_iota[:])
            t_id = cpool.tile([P, P], f32)
            nc.sync.dma_start(out=t_id[:], in_=d_id[:])
            t_ones = cpool.tile([1, P], f32)
            nc.vector.memset(t_ones[:1], 1.0)
            t_eps = cpool.tile([P, 1], f32)
            nc.vector.memset(t_eps[:], EPS)

            ebases = []
            _eb = 0
            for w in range(NW):
                ebases.append(_eb)
                _eb += C[w]

            def emit_seg(w):
                Cw = C[w]
                ebase = ebases[w]

                # ---- segment-sum of this window's edges ----
                # host layout: window block contiguous per partition line:
                # lane p holds rows {c*P+p} for c in [0,Cw)
                e_tile = epool.tile([P, Cmax, EHALVES, HID], edt, tag="ed")
                nc.sync.dma_start(
                    out=e_tile[:, :Cw, :, :],
                    in_=d_ea[ebase * P * EHALVES * HID:
                             (ebase + Cw) * P * EHALVES * HID].rearrange(
                        "(p f) -> p f", p=P),
                )
                t_dstl = spool.tile([P, Cmax4], f32, tag="dstl")
                nc.sync.dma_start(
                    out=t_dstl[:, :Cw],
                    in_=d_dstl[ebase * P:(ebase + Cw) * P].rearrange(
                        "(p c) -> p c", p=P),
                )
                if Cw % OHK:
                    # pad the dstl columns so 4-wide one-hot ops read -1s
                    nc.vector.memset(t_dstl[:, Cw:Cmax4], -1.0)

                ps_agg = aggps.tile([P, HID], f32)
                if OHK == 1:
                    for c in range(Cw):
                        oh = ohpool.tile([P, P], edt, tag="oh")
                        nc.vector.tensor_scalar(
                            out=oh[:], in0=t_iota[:, 0, :],
                            scalar1=t_dstl[:, c:c + 1], scalar2=None,
                            op0=mybir.AluOpType.is_equal,
                        )
                        for hv in range(EHALVES):
                            nc.tensor.matmul(
                                ps_agg[:], lhsT=oh[:],
                                rhs=e_tile[:, c, hv, :],
                                start=(c == 0 and hv == 0),
                                stop=(c == Cw - 1 and hv == EHALVES - 1))
                else:
                    for c0 in range(0, Cw, OHK):
                        k = min(OHK, Cw - c0)
                        oh = ohpool.tile([P, OHK, P], edt, tag="oh")
                        dstl_b = t_dstl[:, c0:c0 + OHK]
                        dstl_b = bass.AP(tensor=dstl_b.tensor,
                                         offset=dstl_b.offset,
                                         ap=list(dstl_b.ap) + [[0, P]])
                        nc.vector.tensor_tensor(
                            out=oh[:], in0=t_iota[:], in1=dstl_b,
                            op=mybir.AluOpType.is_equal,
                        )
                        for i in range(k):
                            c = c0 + i
                            for hv in range(EHALVES):
                                nc.tensor.matmul(
                                    ps_agg[:], lhsT=oh[:, i, :],
                                    rhs=e_tile[:, c, hv, :],
                                    start=(c == 0 and hv == 0),
                                    stop=(c == Cw - 1 and hv == EHALVES - 1))

                return ps_agg

            def emit_node(w, ps_agg):
                ns = slice(w * P, (w + 1) * P)
                # ---- concat [x | agg | u_b] ----
                cat = catpool.tile([P, CAT], f32)
                nc.sync.dma_start(out=cat[:, 0:V_IN], in_=d_x[ns, :])
                nc.scalar.copy(out=cat[:, V_IN:V_IN + HID], in_=ps_agg[:])
                nc.sync.dma_start(out=cat[:, V_IN + HID:CAT], in_=d_ub[ns, :])

                # ---- LayerNorm stats -> xhat ----
                stats = spool.tile([P, 6], f32, tag="st")
                nc.vector.bn_stats(out=stats[:], in_=cat[:])
                mv = spool.tile([P, 2], f32, tag="mv")
                nc.vector.bn_aggr(out=mv[:], in_=stats[:])
                rstd = spool.tile([P, 1], f32, tag="rstd")
                nc.scalar.activation(out=rstd[:], in_=mv[:, 1:2],
                                     func=mybir.ActivationFunctionType.Sqrt,
                                     bias=t_eps[:, :1], scale=1.0)
                nc.vector.reciprocal(out=rstd[:], in_=rstd[:])
                xhat = catpool.tile([P, CAT], f32, tag="xhat")
                nc.vector.tensor_scalar(
                    out=xhat[:], in0=cat[:], scalar1=mv[:, 0:1],
                    scalar2=rstd[:, :1],
                    op0=mybir.AluOpType.subtract, op1=mybir.AluOpType.mult,
                )

                # ---- transpose xhat -> xT chunks ----
                xT = xtpool.tile([P, 3, P], f32)
                for j, off, K in KCH:
                    ptr = trps.tile([P, P], f32, tag="tr")
                    nc.tensor.transpose(out=ptr[:K, :], in_=xhat[:, off:off + K],
                                        identity=t_id[:])
                    nc.scalar.copy(out=xT[:K, j, :], in_=ptr[:K, :])

                # ---- fused [scores | h_partial] = xhat @ [MkgT | W1g] ----
                ps_sh = mmps.tile([P, 3 * P], f32, tag="mm")
                for j, off, K in KCH:
                    nc.tensor.matmul(ps_sh[:, 0:2 * P],
                                     lhsT=xT[:K, j, :],
                                     rhs=t_mw[:K, j, :],
                                     start=(j == 0), stop=(j == 2))
                if not sb_zero:
                    nc.tensor.matmul(ps_sh[:, 0:P], lhsT=t_ones[:1, :],
                                     rhs=t_sb[:1, :], start=False, stop=True,
                                     skip_group_check=True)

                # ---- softmax over MEM (scores half) ----
                negmax = spool.tile([P, 1], f32, tag="nm")
                nc.vector.tensor_reduce(out=negmax[:], in_=ps_sh[:, 0:P],
                                        axis=mybir.AxisListType.X,
                                        op=mybir.AluOpType.max, negate=True)
                pt = wpool.tile([P, MEM], f32, tag="pt")
                ssum = spool.tile([P, 1], f32, tag="ss")
                nc.scalar.activation(out=pt[:], in_=ps_sh[:, 0:P],
                                     func=mybir.ActivationFunctionType.Exp,
                                     bias=negmax[:, :1], scale=1.0,
                                     accum_out=ssum[:, :1])
                rs = spool.tile([P, 1], f32, tag="rs")
                nc.vector.reciprocal(out=rs[:], in_=ssum[:])
                nc.vector.tensor_scalar(out=pt[:], in0=pt[:], scalar1=rs[:, :1],
                                        scalar2=None, op0=mybir.AluOpType.mult)

                # ---- attn transpose ----
                ptr2 = trps.tile([P, P], f32, tag="tr")
                nc.tensor.transpose(out=ptr2[:], in_=pt[:], identity=t_id[:])
                aT = wpool.tile([P, P], f32, tag="aT")
                nc.scalar.copy(out=aT[:], in_=ptr2[:])

                # ---- h = relu(h_partial + attn @ Mv1')  (b1p inside Mv1') ----
                nc.tensor.matmul(ps_sh[:, P:2 * P], lhsT=aT[:], rhs=t_mv1[:],
                                 start=False, stop=True, skip_group_check=True)
                h = wpool.tile([P, HID], f32, tag="h")
                nc.scalar.activation(out=h[:], in_=ps_sh[:, P:2 * P],
                                     func=mybir.ActivationFunctionType.Relu)

                # ---- y = h @ W2 + b2 ----
                ptr3 = trps.tile([P, P], f32, tag="tr")
                nc.tensor.transpose(out=ptr3[:], in_=h[:], identity=t_id[:])
                hT = wpool.tile([P, P], f32, tag="hT")
                nc.scalar.copy(out=hT[:], in_=ptr3[:])
                ps_y = ps_sh[:, 2 * P:3 * P]
                nc.tensor.matmul(ps_y, lhsT=hT[:], rhs=t_w2[:],
                                 start=True, stop=b2_zero,
                                 skip_group_check=True)
                if not b2_zero:
                    nc.tensor.matmul(ps_y, lhsT=t_ones[:1, :],
                                     rhs=t_b2[:1, :], start=False, stop=True,
                                     skip_group_check=True)
                yt = wpool.tile([P, HID], f32, tag="yt")
                nc.scalar.copy(out=yt[:], in_=ps_y)
                nc.sync.dma_start(out=d_out[ns, :], in_=yt[:])

            # software pipeline: stay one window ahead on the segment-sum
            pending = None
            for w in range(NW):
                agg = emit_seg(w)
                if pending is not None:
                    emit_node(w - 1, pending)
                pending = agg
            emit_node(NW - 1, pending)

    if split_waits:
        _split_multi_waits(nc)
    return nc


def _prepare(x, edge_index, edge_attr, u, batch, Mk, Mv, ln_gamma, ln_beta,
             W1, b1, W2, b2):
    """Host-side sharding / packing. Returns (C, in_maps)."""
    x = np.asarray(x, dtype=np.float32)
    edge_attr = np.asarray(edge_attr, dtype=np.float32)
    u = np.asarray(u, dtype=np.float32)
    Mk = np.asarray(Mk, dtype=np.float32)
    Mv = np.asarray(Mv, dtype=np.float32)
    g = np.asarray(ln_gamma, dtype=np.float32)
    be = np.asarray(ln_beta, dtype=np.float32)
    W1 = np.asarray(W1, dtype=np.float32)
    b1 = np.asarray(b1, dtype=np.float32)
    W2 = np.asarray(W2, dtype=np.float32)
    b2 = np.asarray(b2, dtype=np.float32)
    dst = np.asarray(edge_index)[1].astype(np.int64)
    batch = np.asarray(batch).astype(np.int64)

    core_id = dst // N_LOC
    rem = dst - core_id * N_LOC
    w_id = rem >> 7
    loc = (rem & 127).astype(np.float32)
    key = core_id * NW + w_id
    order = np.argsort(key, kind="stable")
    counts = np.bincount(key, minlength=NCORES * NW).reshape(NCORES, NW)
    C = np.maximum((counts.max(axis=0) + P - 1) // P, 1).astype(np.int64)
    E_pad = int(C.sum()) * P
    pad_base = np.concatenate([[0], np.cumsum(C[:-1])]) * P

    starts = np.concatenate([[0], np.cumsum(counts.reshape(-1))])
    loc_sorted = loc[order]

    # per-core edge payload, window-blocked and lane-transposed so each
    # window is ONE contiguous [P, Cw*HID] DMA
    import ml_dtypes
    edt = ml_dtypes.bfloat16 if EDGE_BF16 else np.float32
    ea_pad = np.zeros((NCORES, E_pad * EHALVES * HID), dtype=edt)
    dstl_t = np.full((NCORES, E_pad), -1.0, dtype=np.float32)
    for c in range(NCORES):
        for w in range(NW):
            k = c * NW + w
            s, e = starts[k], starts[k + 1]
            cnt = e - s
            Cw = int(C[w])
            base = pad_base[w]
            blkf = np.zeros((Cw * P, HID), dtype=np.float32)
            blkf[:cnt] = edge_attr[order[s:e]]
            if EDGE_MODE == "split":
                hi = blkf.astype(edt)
                lo = (blkf - hi.astype(np.float32)).astype(edt)
                blk = np.stack([hi.reshape(Cw, P, HID),
                                lo.reshape(Cw, P, HID)], axis=2)
                ea_pad[c, base * 2 * HID:(base + Cw * P) * 2 * HID] = (
                    blk.transpose(1, 0, 2, 3).reshape(-1))
            else:
                blk = blkf.astype(edt)
                ea_pad[c, base * HID:(base + Cw * P) * HID] = (
                    blk.reshape(Cw, P, HID).transpose(1, 0, 2).reshape(-1))
            lb = np.full(Cw * P, -1.0, dtype=np.float32)
            lb[:cnt] = loc_sorted[s:e]
            dstl_t[c, base:base + Cw * P] = lb.reshape(Cw, P).T.reshape(-1)

    u_b = u[batch]
    x_pad = np.zeros((NCORES, N_PAD, V_IN), dtype=np.float32)
    ub_pad = np.zeros((NCORES, N_PAD, U_IN), dtype=np.float32)
    x_pad[:, :N_LOC] = x.reshape(NCORES, N_LOC, V_IN)
    ub_pad[:, :N_LOC] = u_b.reshape(NCORES, N_LOC, U_IN)

    mkgt = (Mk * g[None, :]).T                                   # [CAT, MEM]
    sb = (Mk @ be).reshape(1, MEM)
    w1g = (1.0 - ALPHA) * g[:, None] * W1                        # [CAT, HID]
    mw = np.ascontiguousarray(np.concatenate([mkgt, w1g], axis=1))
    b1p = (1.0 - ALPHA) * (be @ W1) + b1
    mv1 = np.ascontiguousarray(ALPHA * (Mv @ W1) + b1p[None, :])
    b2r = b2.reshape(1, HID)
    iota = np.tile(np.arange(P, dtype=np.float32).astype(edt), (P, OHK, 1))
    ident = np.eye(P, dtype=np.float32)
    key = (tuple(int(v) for v in C),
           bool(np.all(sb == 0.0)), bool(np.all(b2r == 0.0)))

    in_maps = []
    for c in range(NCORES):
        in_maps.append({
            "ea": ea_pad[c], "dstl": dstl_t[c],
            "x": x_pad[c], "ub": ub_pad[c],
            "mw": mw, "mv1": mv1, "w2": W2,
            "sb": sb, "b2": b2r,
            "iota": iota, "ident": ident,
        })
    return key, in_maps


def kernel(**inputs):
    from concourse import bass_utils

    key, in_maps = _prepare(**inputs)
    nc = _nc_cache.get(key)
    if nc is None:
        nc = _build(key)
        _nc_cache[key] = nc
    res = bass_utils.run_bass_kernel_spmd(nc, in_maps, core_ids=list(range(NCORES)))
    out = np.concatenate([r["out"][:N_LOC] for r in res.results], axis=0)
    return out.astype(np.float32)



# revision 30
# speedup vs baseline: 2.4915x; 2.4915x over previous
"""Trainium2 Bass kernel for nn_NodeModel (GNN message passing + external
attention + MLP), SPMD across 8 NeuronCores.

Sharding: nodes (and their incoming edges) are partitioned by destination
across the 8 cores; small params are replicated.

Key ideas vs the naive formulation:
  * Node permutation (degree-sorted snake deal) balances edge counts across
    every (core, window, band) so one shared module structure fits all cores.
  * Segment-sum runs transposed: edge payload [128e, HID] is the STATIONARY
    matmul operand, a narrow banded one-hot [128e, W<=15] is the MOVING one,
    accumulating aggT [HID, nodes] in PSUM. PE cost ~= nodes, not edges.
  * LayerNorm is folded into the attention matmul: mean rides as an augmented
    321st feature row of catT, rstd folds into the activation `scale` of the
    exp/relu, and softmax max-subtraction is a constant shift (exp(x-64))
    which is exact after renormalization.
  * Edge payload is a single bf16 copy (~4e-3 final rel err, gate 2e-2).
  * The fused scores|h matmul runs float32r (1cy/row at free>=256, fp32 bits).
  * Windows are processed in groups of 7 to batch DMAs; outputs leave in
    transposed [HID, nodes] bf16 layout (host untransposes).
"""

import sys

if "/opt/trn_rl_repo" not in sys.path:
    sys.path.insert(0, "/opt/trn_rl_repo")

import numpy as np

N, E, V_IN, HID, U_IN, B, MEM = 50000, 800000, 128, 128, 64, 64, 128
CAT = V_IN + HID + U_IN  # 320
ALPHA = 0.5
EPS = 1e-5
NCORES = 8
P = 128
N_LOC = N // NCORES        # 6250 nodes per core
NW = 49                    # windows of 128 nodes
N_PAD = NW * P             # 6272
NG = 7                     # windows per group
NGRP = NW // NG            # 7 groups
BANDS = (15, 15, 15, 15, 15, 15, 15, 15, 8)   # node counts per band (sum=128)
NBAND = len(BANDS)
BAND_BASE = tuple(int(v) for v in np.cumsum((0,) + BANDS[:-1]))
WMAX = max(BANDS)
SHIFT = 64.0               # constant softmax shift; exact after renorm


# ---------------------------------------------------------------------------
# Workarounds for this container's walrus: at most ONE sync wait per
# instruction is encodable. Tile's scheduler emits multi-waits; split them
# onto same-engine NoOps. Same for the TileContext exit drain.
# ---------------------------------------------------------------------------

def _patched_drain_and_barrier(self, tick_clock, wait_clock):
    from concourse.vector_clock import ScopedClock, VectorClock

    nc = self.nc
    gvc = tick_clock.global_clock
    nprocs = len(gvc)
    for proc in range(nprocs):
        tick = gvc[proc]
        if tick <= 0:
            continue
        one = VectorClock([0] * nprocs)
        one.require_at_least(proc, tick)
        inst = nc.sync.drain()
        wait_clock.add_sem_waits(inst.ins, ScopedClock({None: one}))
    nc.sync.drain()
    nc.all_engine_barrier()
    assert self.sems is not None
    popped = nc._tile_sem_poison_stack.pop()
    assert popped is self._sem_poison
    nc.clear_and_free_semaphores(list(self.sems.allocated().values()))
    nc.all_engine_barrier()


def _split_multi_waits(nc):
    from concourse import mybir

    for f in nc.m.functions:
        for bb in f.blocks:
            out = []
            for inst in bb.instructions:
                si = inst.sync_info
                if si is not None and si.on_wait is not None and len(si.on_wait) > 1:
                    waits = list(si.on_wait)
                    for i, w in enumerate(waits[:-1]):
                        out.append(mybir.InstNoOp(
                            name=f"{inst.name}-wsplit{i}",
                            engine=inst.engine,
                            sync_info=mybir.SyncInfo(on_wait=[w], on_update=[]),
                        ))
                    si.on_wait = waits[-1:]
                out.append(inst)
            bb.instructions[:] = out


_patch_applied = False


def _apply_patches():
    global _patch_applied
    if _patch_applied:
        return
    import concourse.tile as tile

    tile.TileContext._drain_and_barrier = _patched_drain_and_barrier
    _patch_applied = True


# ---------------------------------------------------------------------------
# Bass module builder. Kernel structure depends only on per-(window, band)
# chunk counts C (shared across cores) and the zero-flags, so cache on that.
# ---------------------------------------------------------------------------

_nc_cache = {}


def _build(key, split_waits=True):
    """key: (C, sb_zero, b2_zero); C[w][b] = 128-edge chunks per band."""
    import concourse.bass as bass
    import concourse.tile as tile
    from concourse import mybir

    C, sb_zero, b2_zero = key
    _apply_patches()
    f32 = mybir.dt.float32
    f32r = mybir.dt.float32r
    bf16 = mybir.dt.bfloat16

    CW = [sum(C[w]) for w in range(NW)]          # chunks per window
    CWmax = max(CW)
    # flat slot offsets (in 128-edge chunks) per window
    woff = [0]
    for w in range(NW):
        woff.append(woff[-1] + CW[w])
    TOTCH = woff[-1]
    # group chunk offsets for dstl
    goff = [woff[g * NG] for g in range(NGRP)] + [TOTCH]
    GN = NG * P                                   # nodes per group (896)
    KCH = [(0, 128), (1, 128), (2, 64)]           # stats contraction chunks

    nc = bass.Bass()
    d_ea = nc.dram_tensor("ea", [TOTCH * P * HID], bf16, kind="ExternalInput")
    d_dstl = nc.dram_tensor("dstl", [TOTCH * P], bf16, kind="ExternalInput")
    d_xt = nc.dram_tensor("xt", [V_IN, N_PAD], f32r, kind="ExternalInput")
    d_ubt = nc.dram_tensor("ubt", [U_IN, N_PAD], f32r, kind="ExternalInput")
    d_mw = nc.dram_tensor("mw", [P, 3 * 2 * P], f32r, kind="ExternalInput")
    d_mv1 = nc.dram_tensor("mv1", [MEM, HID], bf16, kind="ExternalInput")
    d_w2 = nc.dram_tensor("w2", [HID, HID], bf16, kind="ExternalInput")
    d_iota = nc.dram_tensor("iota", [P, WMAX], bf16, kind="ExternalInput")
    d_idb = nc.dram_tensor("identb", [P, P], bf16, kind="ExternalInput")
    d_ones = nc.dram_tensor("ones", [P, 4], f32r, kind="ExternalInput")
    d_sb = nc.dram_tensor("sb", [1, 2 * P], bf16, kind="ExternalInput")
    d_b2 = nc.dram_tensor("b2", [1, P], bf16, kind="ExternalInput")
    d_out = nc.dram_tensor("out", [HID, N_PAD], bf16, kind="ExternalOutput")

    with tile.TileContext(nc) as tc:
        with (
            tc.tile_pool(name="const", bufs=1) as cpool,
            tc.tile_pool(name="edges", bufs=4) as epool,
            tc.tile_pool(name="dstl", bufs=2) as dpool,
            tc.tile_pool(name="oh", bufs=3) as ohpool,
            tc.tile_pool(name="cat", bufs=2) as catpool,
            tc.tile_pool(name="sq", bufs=2) as sqpool,
            tc.tile_pool(name="stat", bufs=2) as stpool,
            tc.tile_pool(name="work", bufs=3) as wpool,
            tc.tile_pool(name="outb", bufs=2) as opool,
            tc.tile_pool(name="agg_ps", bufs=2, space="PSUM") as aggps,
            tc.tile_pool(name="sh_ps", bufs=2, space="PSUM") as shps,
            tc.tile_pool(name="st_ps", bufs=2, space="PSUM") as stps,
            tc.tile_pool(name="tr_ps", bufs=2, space="PSUM") as trps,
        ):
            # ---- constants ----
            t_mw = cpool.tile([P, 3, 2 * P], f32r)
            nc.sync.dma_start(out=t_mw[:], in_=d_mw[:])
            t_mv1 = cpool.tile([MEM, HID], bf16)
            nc.sync.dma_start(out=t_mv1[:], in_=d_mv1[:])
            t_w2 = cpool.tile([HID, HID], bf16)
            nc.sync.dma_start(out=t_w2[:], in_=d_w2[:])
            t_iota = cpool.tile([P, WMAX], bf16)
            nc.sync.dma_start(out=t_iota[:], in_=d_iota[:])
            t_idb = cpool.tile([P, P], bf16)
            nc.sync.dma_start(out=t_idb[:], in_=d_idb[:])
            t_ones = cpool.tile([P, 4], f32r)
            nc.sync.dma_start(out=t_ones[:], in_=d_ones[:])
            t_onesb = cpool.tile([P, 1], bf16)
            nc.vector.memset(t_onesb[:], 1.0)
            t_eps = cpool.tile([P, 1], f32)
            nc.vector.memset(t_eps[:], EPS)
            t_shift = cpool.tile([P, 1], f32)
            nc.vector.memset(t_shift[:], -SHIFT)
            if not sb_zero:
                t_sb = cpool.tile([1, 2 * P], bf16)
                nc.sync.dma_start(out=t_sb[:1], in_=d_sb[:])
            if not b2_zero:
                t_b2 = cpool.tile([1, P], bf16)
                nc.sync.dma_start(out=t_b2[:1], in_=d_b2[:])
                t_onr = cpool.tile([1, P], bf16)
                nc.vector.memset(t_onr[:1], 1.0)

            groups = {}

            def group_load(g):
                gd = {}
                gd["cat"] = catpool.tile([P, 3, GN], f32r, tag="cat", name="catT")
                ns0, ns1 = g * GN, (g + 1) * GN
                nc.sync.dma_start(out=gd["cat"][:, 0, :], in_=d_xt[:, ns0:ns1])
                nc.sync.dma_start(out=gd["cat"][0:U_IN, 2, :],
                                  in_=d_ubt[:, ns0:ns1])
                gcwmax = max(goff[i + 1] - goff[i] for i in range(NGRP))
                gd["dstl"] = dpool.tile([P, gcwmax], bf16, tag="dstl", name="dstl")
                gcw = goff[g + 1] - goff[g]
                nc.sync.dma_start(
                    out=gd["dstl"][:, :gcw],
                    in_=d_dstl[goff[g] * P:goff[g + 1] * P].rearrange(
                        "(p f) -> p f", p=P),
                )
                gd["out"] = opool.tile([P, GN], bf16, tag="out", name="outb")
                groups[g] = gd
                return gd

            def emit_seg(ns):
                g, wi = divmod(ns, NG)
                gd = groups[g]
                cw = CW[ns]
                e_tile = epool.tile([P, CWmax, HID], bf16, tag="ed")
                nc.sync.dma_start(
                    out=e_tile[:, :cw, :],
                    in_=d_ea[woff[ns] * P * HID:woff[ns + 1] * P * HID]
                        .rearrange("(p f) -> p f", p=P),
                )
                # one-hot for all chunks of this window in ONE DVE op:
                # oh[p, c, j] = (iota[j] == dstl[p, c]); padding lanes hold -1.
                oh = ohpool.tile([P, CWmax, WMAX], bf16, tag="oh")
                io = t_iota[:, :WMAX]
                io_b = bass.AP(tensor=io.tensor, offset=io.offset,
                               ap=[io.ap[0], [0, cw], io.ap[1]])
                dloc = gd["dstl"][:, woff[ns] - goff[g]:woff[ns + 1] - goff[g]]
                dloc_b = bass.AP(tensor=dloc.tensor, offset=dloc.offset,
                                 ap=list(dloc.ap) + [[0, WMAX]])
                nc.vector.tensor_tensor(out=oh[:, :cw, :], in0=io_b,
                                        in1=dloc_b,
                                        op=mybir.AluOpType.is_equal)

                ps_aggT = aggps.tile([HID, P], f32, tag="agg")
                ci = 0
                for b in range(NBAND):
                    nb = C[ns][b]
                    base, wid = BAND_BASE[b], BANDS[b]
                    for k in range(nb):
                        nc.tensor.matmul(
                            ps_aggT[:, base:base + wid],
                            lhsT=e_tile[:, ci, :],
                            rhs=oh[:, ci, :wid],
                            start=(k == 0), stop=(k == nb - 1),
                            skip_group_check=True)
                        ci += 1
                # aggT -> catT chunk 1 (Pool engine, fp32->fp32)
                nc.vector.tensor_copy(gd["cat"][:, 1, wi * P:(wi + 1) * P],
                                      ps_aggT[:])

            def emit_stats(g):
                gd = groups[g]
                cat = gd["cat"]
                sq = sqpool.tile([P, 3, GN], bf16, tag="sq")
                nc.scalar.activation(out=sq[:, 0:2, :], in_=cat[:, 0:2, :],
                                     func=mybir.ActivationFunctionType.Square)
                nc.scalar.activation(out=sq[0:64, 2, :], in_=cat[0:64, 2, :],
                                     func=mybir.ActivationFunctionType.Square)
                ps_st = stps.tile([P, NG, 4], f32, tag="st")
                for wi in range(NG):
                    ws = slice(wi * P, (wi + 1) * P)
                    for j, K in KCH:
                        nc.tensor.matmul(
                            ps_st[:, wi, 0:2],
                            lhsT=cat[0:K, j, ws],
                            rhs=t_ones[0:K, 0:2],
                            start=(j == 0), stop=(j == 2),
                            skip_group_check=True)
                    for j, K in KCH:
                        nc.tensor.matmul(
                            ps_st[:, wi, 2:3],
                            lhsT=sq[0:K, j, ws],
                            rhs=t_onesb[0:K, 0:1],
                            start=(j == 0), stop=(j == 2),
                            skip_group_check=True)
                # group stat tiles
                st_sb = stpool.tile([P, NG, 4], f32, tag="stsb")
                nc.vector.tensor_scalar(out=st_sb[:, :, 0:3],
                                        in0=ps_st[:, :, 0:3],
                                        scalar1=1.0 / CAT, scalar2=None,
                                        op0=mybir.AluOpType.mult)
                mu2 = stpool.tile([P, NG], f32, tag="mu2")
                nc.vector.tensor_tensor(out=mu2[:], in0=st_sb[:, :, 0],
                                        in1=st_sb[:, :, 0],
                                        op=mybir.AluOpType.mult)
                var = stpool.tile([P, NG], f32, tag="var")
                nc.vector.tensor_tensor(out=var[:], in0=st_sb[:, :, 2],
                                        in1=mu2[:],
                                        op=mybir.AluOpType.subtract)
                std = stpool.tile([P, NG], f32, tag="std")
                nc.scalar.activation(out=std[:], in_=var[:],
                                     func=mybir.ActivationFunctionType.Sqrt,
                                     bias=t_eps[:, 0:1], scale=1.0)
                rstd = stpool.tile([P, NG], f32, tag="rstd")
                nc.vector.reciprocal(out=rstd[:], in_=std[:])
                gd["std"] = std
                gd["rstd"] = rstd
                gd["ssum"] = stpool.tile([P, NG], f32, tag="ssum", name="ssum")
                gd["rs"] = stpool.tile([P, NG], f32, tag="rs", name="rs")


            def emit_node(ns):
                g, wi = divmod(ns, NG)
                gd = groups[g]
                cat = gd["cat"]
                rstd = gd["rstd"]
                ws = slice(wi * P, (wi + 1) * P)
                ps_sh = shps.tile([P, 3 * P], f32, tag="sh")
                for j, K in KCH:
                    nc.tensor.matmul(
                        ps_sh[:, 0:2 * P],
                        lhsT=cat[0:K, j, ws],
                        rhs=t_mw[0:K, j, :],
                        start=(j == 0), stop=(j == 2),
                        skip_group_check=True)
                if not sb_zero:
                    # scores += (1/rstd)_n * sb[m]  (rank-1, rare path)
                    stdb = wpool.tile([P, 1], bf16, tag="stdb")
                    nc.vector.tensor_copy(stdb[:], gd["std"][:, wi:wi + 1])
                    ps_rvt = trps.tile([P, 2 * P], bf16, tag="tr",
                                       name="ps_rvt")
                    ps_rv = ps_rvt[0:1, 0:P]
                    nc.tensor.transpose(out=ps_rv, in_=stdb[:],
                                        identity=t_idb[:])
                    rvr = wpool.tile([1, P], bf16, tag="rvr")
                    nc.vector.tensor_copy(rvr[:1], ps_rv)
                    nc.tensor.matmul(ps_sh[:, 0:2 * P], lhsT=rvr[:1],
                                     rhs=t_sb[:1], start=False, stop=True,
                                     skip_group_check=True)
                # softmax: exp(rstd*score - SHIFT), renormalized later
                pt = wpool.tile([P, MEM], bf16, tag="pt")
                nc.scalar.activation(out=pt[:], in_=ps_sh[:, 0:P],
                                     func=mybir.ActivationFunctionType.Exp,
                                     bias=t_shift[:, 0:1], scale=rstd[:, wi:wi + 1],
                                     accum_out=gd["ssum"][:, wi:wi + 1])
                # pt2 = pt * (1/ssum) * std  (normalize softmax, pre-div rstd)
                nc.vector.reciprocal(out=gd["rs"][:, wi:wi + 1],
                                     in_=gd["ssum"][:, wi:wi + 1])
                pt2 = wpool.tile([P, MEM], bf16, tag="pt2")
                nc.vector.tensor_scalar(out=pt2[:], in0=pt[:],
                                        scalar1=gd["rs"][:, wi:wi + 1],
                                        scalar2=gd["std"][:, wi:wi + 1],
                                        op0=mybir.AluOpType.mult,
                                        op1=mybir.AluOpType.mult)
                ps_ptTt = trps.tile([P, 2 * P], bf16, tag="tr",
                                    name="ps_ptTt")
                ps_ptT = ps_ptTt[:, 0:P]
                nc.tensor.transpose(out=ps_ptT, in_=pt2[:],
                                    identity=t_idb[:])
                ptT = wpool.tile([P, P], bf16, tag="ptT")
                nc.vector.tensor_copy(ptT[:], ps_ptT)
                nc.tensor.matmul(ps_sh[:, P:2 * P], lhsT=ptT[:], rhs=t_mv1[:],
                                 start=False, stop=True, skip_group_check=True)
                h = wpool.tile([P, HID], bf16, tag="h")
                nc.scalar.activation(out=h[:], in_=ps_sh[:, P:2 * P],
                                     func=mybir.ActivationFunctionType.Relu,
                                     scale=rstd[:, wi:wi + 1])
                ps_hTt = trps.tile([P, 2 * P], bf16, tag="tr",
                                   name="ps_hTt")
                ps_hT = ps_hTt[:, 0:P]
                nc.tensor.transpose(out=ps_hT, in_=h[:], identity=t_idb[:])
                hT = wpool.tile([P, P], bf16, tag="hT")
                nc.vector.tensor_copy(hT[:], ps_hT)
                ps_y = ps_sh[:, 2 * P:3 * P]
                nc.tensor.matmul(ps_y, lhsT=t_w2[:], rhs=hT[:],
                                 start=True, stop=b2_zero,
                                 skip_group_check=True)
                if not b2_zero:
                    nc.tensor.matmul(ps_y, lhsT=t_b2[:1], rhs=t_onr[:1],
                                     start=False, stop=True,
                                     skip_group_check=True)
                nc.scalar.copy(out=gd["out"][:, ws], in_=ps_y)

            def emit_out(g):
                gd = groups.pop(g)
                nc.sync.dma_start(out=d_out[:, g * GN:(g + 1) * GN],
                                  in_=gd["out"][:])

            # software pipeline: one group ahead on load+seg
            for ns in range(NW):
                g, wi = divmod(ns, NG)
                if wi == 0:
                    group_load(g)
                emit_seg(ns)
                if wi == NG - 1:
                    emit_stats(g)
                if ns >= NG:
                    emit_node(ns - NG)
                    if (ns - NG) % NG == NG - 1:
                        emit_out((ns - NG) // NG)
            for ns in range(NW - NG, NW):
                emit_node(ns)
            emit_out(NGRP - 1)

    if split_waits:
        _split_multi_waits(nc)
    return nc


# ---------------------------------------------------------------------------
# Host-side sharding / packing.
# ---------------------------------------------------------------------------

def _prepare(x, edge_index, edge_attr, u, batch, Mk, Mv, ln_gamma, ln_beta,
             W1, b1, W2, b2):
    import ml_dtypes
    bf16 = ml_dtypes.bfloat16

    x = np.asarray(x, dtype=np.float32)
    edge_attr = np.asarray(edge_attr, dtype=np.float32)
    u = np.asarray(u, dtype=np.float32)
    Mk = np.asarray(Mk, dtype=np.float32)
    Mv = np.asarray(Mv, dtype=np.float32)
    g = np.asarray(ln_gamma, dtype=np.float32)
    be = np.asarray(ln_beta, dtype=np.float32)
    W1 = np.asarray(W1, dtype=np.float32)
    b1 = np.asarray(b1, dtype=np.float32)
    W2 = np.asarray(W2, dtype=np.float32)
    b2 = np.asarray(b2, dtype=np.float32)
    dst = np.asarray(edge_index)[1].astype(np.int64)
    batch = np.asarray(batch).astype(np.int64)

    # --- node permutation: degree-ASC snake deal over (core, window, band) ---
    deg = np.bincount(dst, minlength=N)
    order_nodes = np.argsort(deg, kind="stable")
    ninst = NCORES * NW * NBAND
    inst_band = np.tile(np.arange(NBAND), NCORES * NW)
    # slot sequence: for each pass p, snake over instances with capacity > p
    node_core = np.empty(N, dtype=np.int32)
    node_win = np.empty(N, dtype=np.int32)       # window within core
    node_pos = np.empty(N, dtype=np.int32)       # position within window
    bands_arr = np.array(BANDS)
    base_arr = np.array(BAND_BASE)
    k = 0
    for p in range(WMAX):
        live = np.nonzero(bands_arr[inst_band] > p)[0]
        if p % 2 == 1:
            live = live[::-1]
        take = min(len(live), N - k)
        if take == 0:
            break
        sel = live[:take]
        nodes = order_nodes[k:k + take]
        node_core[nodes] = sel // (NW * NBAND)
        rem = sel % (NW * NBAND)
        node_win[nodes] = rem // NBAND
        node_pos[nodes] = base_arr[rem % NBAND] + p
        k += take
    assert k == N

    # --- edges: sort by (core, window, band) ---
    e_core = node_core[dst]
    e_win = node_win[dst]
    e_pos = node_pos[dst]
    e_band = np.searchsorted(np.array(BAND_BASE + (128,)), e_pos,
                             side="right") - 1
    e_slot = e_pos - base_arr[e_band]            # 0..W_b-1 (the dstl value)
    e_key = ((e_core * NW + e_win) * NBAND + e_band).astype(np.int64)
    order = np.argsort(e_key, kind="stable")
    counts = np.bincount(e_key, minlength=NCORES * NW * NBAND)
    counts = counts.reshape(NCORES, NW, NBAND)
    C = np.maximum((counts.max(axis=0) + P - 1) // P, 1)   # [NW, NBAND]
    CW = C.sum(axis=1)                                      # chunks per window
    woff = np.concatenate([[0], np.cumsum(CW)])
    TOTCH = int(woff[-1])

    starts = np.concatenate([[0], np.cumsum(counts.reshape(-1))])
    slot_sorted = e_slot[order].astype(np.float32)
    ea16 = edge_attr.astype(bf16)

    ea_pad = np.zeros((NCORES, TOTCH * P * HID), dtype=bf16)
    dstl = np.full((NCORES, TOTCH * P), -1.0, dtype=bf16)
    for c in range(NCORES):
        win_d = []
        for w in range(NW):
            cw = int(CW[w])
            blk = np.zeros((cw * P, HID), dtype=bf16)
            dblk = np.full(cw * P, -1.0, dtype=np.float32)
            coff = 0
            for b in range(NBAND):
                kk = (c * NW + w) * NBAND + b
                s, e = starts[kk], starts[kk + 1]
                cnt = e - s
                blk[coff * P:coff * P + cnt] = ea16[order[s:e]]
                dblk[coff * P:coff * P + cnt] = slot_sorted[s:e]
                coff += int(C[w][b])
            base = int(woff[w])
            ea_pad[c, base * P * HID:(base + cw) * P * HID] = (
                blk.reshape(cw, P, HID).transpose(1, 0, 2).reshape(-1))
            win_d.append(dblk.reshape(cw, P).T)  # [P, cw] lane-major
        # dstl is DMAed per GROUP with one (p f) rearrange, so each group
        # block must be group-lane-major: lane p holds its 7 windows' chunks
        # back-to-back.
        goffs = [int(woff[gi * NG]) for gi in range(NGRP)] + [int(TOTCH)]
        for gi in range(NGRP):
            grp = np.hstack(win_d[gi * NG:(gi + 1) * NG]).astype(bf16)
            dstl[c, goffs[gi] * P:goffs[gi + 1] * P] = grp.reshape(-1)
    # --- node features, transposed + permuted per core ---
    u_b = u[batch]
    xt = np.zeros((NCORES, V_IN, N_PAD), dtype=np.float32)
    ubt = np.zeros((NCORES, U_IN, N_PAD), dtype=np.float32)
    allpos = node_win.astype(np.int64) * P + node_pos
    for c in range(NCORES):
        m = node_core == c
        xt[c][:, allpos[m]] = x[m].T
        ubt[c][:, allpos[m]] = u_b[m].T

    # --- folded params ---
    mkgt = (Mk * g[None, :]).T                         # [CAT, MEM]
    w1g = (1.0 - ALPHA) * g[:, None] * W1              # [CAT, HID]
    mw_full = np.concatenate([mkgt, w1g], axis=1)      # [CAT, 256]
    # LN output sums to zero across features, so centering each column is
    # exact — and it eliminates the mean-correction term entirely.
    mw_full = mw_full - mw_full.sum(axis=0, keepdims=True) / CAT
    mw = np.zeros((P, 3, 2 * P), dtype=np.float32)
    mw[:, 0, :] = mw_full[0:128]
    mw[:, 1, :] = mw_full[128:256]
    mw[0:64, 2, :] = mw_full[256:320]
    b1p = (1.0 - ALPHA) * (be @ W1) + b1
    mv1 = (ALPHA * (Mv @ W1) + b1p[None, :]).astype(bf16)
    sb = np.zeros((1, 2 * P), dtype=np.float32)
    sb[0, 0:MEM] = Mk @ be
    iota = np.tile(np.arange(WMAX, dtype=np.float32).astype(bf16), (P, 1))

    key = (tuple(tuple(int(v) for v in C[w]) for w in range(NW)),
           bool(np.all(sb == 0.0)), bool(np.all(b2 == 0.0)))

    in_maps = []
    for c in range(NCORES):
        in_maps.append({
            "ea": ea_pad[c], "dstl": dstl[c],
            "xt": xt[c], "ubt": ubt[c],
            "mw": mw.reshape(P, 3 * 2 * P), "mv1": mv1,
            "w2": W2.astype(bf16),
            "iota": iota, "identb": np.eye(P, dtype=np.float32).astype(bf16),
            "ones": np.ones((P, 4), dtype=np.float32),
            "sb": sb.astype(bf16), "b2": b2.reshape(1, P).astype(bf16),
        })
    unshard = (node_core, allpos)
    return key, in_maps, unshard


def kernel(**inputs):
    from concourse import bass_utils

    key, in_maps, unshard = _prepare(**inputs)
    nc = _nc_cache.get(key)
    if nc is None:
        nc = _build(key)
        _nc_cache[key] = nc
    res = bass_utils.run_bass_kernel_spmd(nc, in_maps,
                                          core_ids=list(range(NCORES)))
    node_core, allpos = unshard
    out = np.empty((N, HID), dtype=np.float32)
    for c in range(NCORES):
        m = node_core == c
        out[m] = np.asarray(res.results[c]["out"]).astype(np.float32)[
            :, allpos[m]].T
    return out
